# revision 16
# baseline (speedup 1.0000x reference)
"""Paged GQA decode attention (1 token/seq) on 8 trn2 NeuronCores.

Sharding: tensor-parallel over the 8 KV heads. Core i owns KV head i and
its G=4 query heads: Wq/Wk/Wv column-sharded, Wo row-sharded; each core
produces a partial [BS, HID] output and the host sums the 8 partials.

All matmul operands are bf16 (weights, KV cache, q/k/v, exp probs); f32
is kept for PSUM accumulation, softmax denominator, RMS/RoPE math, and
the output. The KV cache is repacked host-side in the device's sequence
processing order so that consecutive short sequences coalesce into one
contiguous multi-sequence DMA (<= 32 blocks per group, one contiguous
multi-KB run per partition).

Per-core dataflow (one Bass program, SPMD over cores via per-core inputs):
  A) qkv = hidT.T @ W  (PE, 32 k-tiles accumulated in PSUM); wq streams
     in two halves around wkv so Q-head RMSNorm/RoPE (ACT/DVE) overlaps
     the remaining projection matmuls; attn scale folded into q's
     inverse-rms; q/k transposed to [D, *] via PE transpose.
  B) per sequence, per 128-token tile:
     S^T[tok,4] = matmul(lhsT=KT tile, rhs=qT_b)
     E = exp(S^T) on ACT (tail mask folded into exp's bias operand;
     no max subtraction: q,k RMS-normed => |s|<=~12, bf16 E holds it)
     O^T[130,4] += matmul(lhsT=E, rhs=V_tile); V col 128 is ones so its
     output row is the softmax denominator.
     New-token fix: the stale cache slot at pos=T-1 gets its KT column /
     V row overwritten with this step's k/v before the matmuls.
     Sequence order: 2 longest first (DMA backlog for the pre-compute
     window), short/medium interleave, 4 longest last (compute:DMA < 1
     so the post-DMA tail drains).
  C) partial = attn^T.T @ Wo_slice (PE), DMA'd out per 512-col chunk.
"""

import numpy as np
import ml_dtypes

HID, H, HKV, D = 4096, 32, 8, 128
BS, BLOCKS_PER_SEQ, BLOCK_SIZE = 32, 32, 128
G = H // HKV
EPS = 1e-6
NCORES = 8
KTILES = HID // 128  # contraction tiles for the projections
VW = D + 2  # V row width: 128 data + ones col (denominator) + pad
GROUP_MAX = 32  # max cache blocks per coalesced DMA group

BF16 = ml_dtypes.bfloat16

_prog_cache = {}


def _plan(seq_lens):
    """Processing order, packed block offsets, and DMA groups."""
    nb = [(int(t) + BLOCK_SIZE - 1) // BLOCK_SIZE for t in seq_lens]
    sd = sorted(range(BS), key=lambda b: -nb[b])
    order = [sd[0], sd[1]]
    mid = sd[6:]
    i, j = len(mid) - 1, 0
    while j <= i:
        order.append(mid[i])
        if i != j:
            order.append(mid[j])
        i -= 1
        j += 1
    order.extend([sd[5], sd[4], sd[3], sd[2]])
    # pack offsets follow processing order
    pstart = {}
    off = 0
    for b in order:
        pstart[b] = off
        off += nb[b]
    nbtot = off
    # greedy groups of consecutive seqs, <= GROUP_MAX blocks each
    groups = []
    cur = []
    cnt = 0
    for b in order:
        if cur and cnt + nb[b] > GROUP_MAX:
            groups.append(cur)
            cur, cnt = [], 0
        cur.append(b)
        cnt += nb[b]
    if cur:
        groups.append(cur)
    return nb, order, pstart, nbtot, groups


def _build_program(seq_lens, apply_qw, apply_kw):
    import concourse.bass as bass
    import concourse.tile as tile
    from concourse import bacc, mybir

    f32 = mybir.dt.float32
    bf16 = mybir.dt.bfloat16
    AF = mybir.ActivationFunctionType

    nb, order, pstart, nbtot, groups = _plan(seq_lens)

    nc = bacc.Bacc("TRN2", target_bir_lowering=False)
    hidT = nc.dram_tensor("hidT", [128, KTILES * BS], bf16, kind="ExternalInput")
    wq = nc.dram_tensor("wq", [128, KTILES * G * D], bf16, kind="ExternalInput")
    wkv = nc.dram_tensor("wkv", [128, KTILES * 2 * D], bf16, kind="ExternalInput")
    wo = nc.dram_tensor("wo", [128, G * HID], bf16, kind="ExternalInput")
    # K packed [d, packed_tok]: per-group slice is contiguous per partition
    kp = nc.dram_tensor("kp", [128, nbtot * BLOCK_SIZE], bf16, kind="ExternalInput")
    # V packed [tok_in_block, packed_block*VW + d]; col 128 ones, col 129 zero
    vp = nc.dram_tensor("vp", [128, nbtot * VW], bf16, kind="ExternalInput")
    miscd = nc.dram_tensor("miscd", [128, 512], f32, kind="ExternalInput")
    if apply_qw:
        qw = nc.dram_tensor("qw", [1, D], f32, kind="ExternalInput")
    if apply_kw:
        kw = nc.dram_tensor("kw", [1, D], f32, kind="ExternalInput")
    outp = nc.dram_tensor("outp", [BS, HID], f32, kind="ExternalOutput")

    with tile.TileContext(nc) as tc:
        with tc.tile_pool(name="sb", bufs=1) as sb, tc.tile_pool(
            name="smalls", bufs=4
        ) as smalls:
            HQ = KTILES * 2 * D  # cols per wq half (heads 01 / 23)
            # misc: maskM | ident | cos | sin in one DMA
            misc_sb = sb.tile([128, 512], f32, name="misc_sb")
            nc.sync.dma_start(out=misc_sb, in_=miscd[:, :])
            maskM = misc_sb[:, 0:128]
            ident = misc_sb[:, 128:256]
            cos_sb = misc_sb[:, 256:384]
            sin_sb = misc_sb[:, 384:512]
            hid_sb = sb.tile([128, KTILES * BS], bf16, name="hid_sb")
            nc.sync.dma_start(out=hid_sb, in_=hidT[:, :])

            # weights dual-issued in column halves across both HWDGE queues:
            # each queue is FIFO, so both serve weights before any KV bytes
            # and the weights get the full DMA engine pool
            def wsplit(dst, srcap, cols):
                h = cols // 2
                nc.sync.dma_start(out=dst[:, 0:h], in_=srcap[:, 0:h])
                nc.scalar.dma_start(out=dst[:, h:cols], in_=srcap[:, h:cols])

            wq_sb = sb.tile([128, KTILES * G * D], bf16, name="wq_sb")
            wsplit(wq_sb[:, 0:HQ], wq[:, 0:HQ], HQ)
            wkv_sb = sb.tile([128, KTILES * 2 * D], bf16, name="wkv_sb")
            wsplit(wkv_sb, wkv[:, :], KTILES * 2 * D)
            wsplit(wq_sb[:, HQ : 2 * HQ], wq[:, HQ : 2 * HQ], HQ)
            wo_sb = sb.tile([128, G * HID], bf16, name="wo_sb")

            warm = sb.tile([128, 128], bf16, name="warm")
            nc.vector.memset(warm, 0.0)
            with tc.tile_pool(name="psW", bufs=1, space="PSUM") as psW:
                wps = psW.tile([128, 128], f32, name="wps")
                for t in range(24):
                    nc.tensor.matmul(
                        wps, warm, warm, start=(t == 0), stop=(t == 23)
                    )

            norm_w_sb = {}
            for flag, name, dram in (
                (apply_qw, "qw_sb", qw if apply_qw else None),
                (apply_kw, "kw_sb", kw if apply_kw else None),
            ):
                if flag:
                    t = sb.tile([BS, D], f32, name=name)
                    src = dram[:, :]
                    bcast = bass.AP(
                        tensor=src.tensor,
                        offset=src.offset,
                        ap=[[0, BS], list(src.ap[-1])],
                    )
                    nc.sync.dma_start(out=t, in_=bcast)
                    norm_w_sb[name] = t

            eps_q = sb.tile([BS, 1], f32, name="eps_q")
            nc.vector.memset(eps_q, float(D) * EPS)
            eps_k = sb.tile([BS, 1], f32, name="eps_k")
            nc.vector.memset(eps_k, EPS)

            qr_sb = sb.tile([BS, G * D], f32, name="qr_sb")
            kr_sb = sb.tile([BS, D], f32, name="kr_sb")
            v_sb = sb.tile([BS, D], bf16, name="v_sb")
            qT_sb = sb.tile([128, G * BS], bf16, name="qT_sb")
            kT_sb = sb.tile([128, BS], bf16, name="kT_sb")
            attn_T = sb.tile([128, G * BS], bf16, name="attn_T")

            with tc.tile_pool(name="psA", bufs=1, space="PSUM") as psA:
                kv_ps = psA.tile([BS, 2 * D], f32, name="kv_ps")
                q01_ps = psA.tile([BS, 2 * D], f32, name="q01_ps")
                q23_ps = psA.tile([BS, 2 * D], f32, name="q23_ps")
                last = KTILES - 1
                for t in range(KTILES):
                    lt = hid_sb[:, t * BS : (t + 1) * BS]
                    nc.tensor.matmul(
                        q01_ps, lt, wq_sb[:, t * 2 * D : (t + 1) * 2 * D],
                        start=(t == 0), stop=(t == last),
                    )
                for t in range(KTILES):
                    lt = hid_sb[:, t * BS : (t + 1) * BS]
                    nc.tensor.matmul(
                        kv_ps, lt, wkv_sb[:, t * 2 * D : (t + 1) * 2 * D],
                        start=(t == 0), stop=(t == last),
                    )
                for t in range(KTILES):
                    lt = hid_sb[:, t * BS : (t + 1) * BS]
                    nc.tensor.matmul(
                        q23_ps, lt, wq_sb[:, HQ + t * 2 * D : HQ + (t + 1) * 2 * D],
                        start=(t == 0), stop=(t == last),
                    )
                k_ps = kv_ps[:, 0:D]
                v_ps = kv_ps[:, D : 2 * D]

                def norm_rope(slices, dst, is_q, h0=0):
                    w_sb = norm_w_sb.get("qw_sb" if is_q else "kw_sb")
                    for hh, xin in enumerate(slices):
                        h = h0 + hh
                        scratch = smalls.tile([BS, D], f32, name="scratch", tag="scr")
                        ssq = smalls.tile([BS, 1], f32, name="ssq", tag="ssq")
                        nc.scalar.activation(scratch, xin, AF.Square, accum_out=ssq)
                        s = smalls.tile([BS, 1], f32, name="s", tag="s")
                        if is_q:
                            # s = sqrt(sum(q^2) + D*eps): 1/s folds in the
                            # attention scale D**-0.5 on top of the rms norm
                            nc.scalar.activation(s, ssq, AF.Sqrt, bias=eps_q, scale=1.0)
                        else:
                            nc.scalar.activation(s, ssq, AF.Sqrt, bias=eps_k, scale=1.0 / D)
                        inv = smalls.tile([BS, 1], f32, name="inv", tag="inv")
                        nc.vector.reciprocal(inv, s)
                        xn = smalls.tile([BS, D], f32, name="xn", tag="xn")
                        nc.scalar.activation(xn, xin, AF.Copy, scale=inv)
                        if w_sb is not None:
                            nc.vector.tensor_mul(xn, xn, w_sb)
                        rot = smalls.tile([BS, D], f32, name="rot", tag="rot")
                        nc.scalar.mul(rot[:, 0 : D // 2], xn[:, D // 2 : D], -1.0)
                        nc.scalar.copy(rot[:, D // 2 : D], xn[:, 0 : D // 2])
                        t1 = smalls.tile([BS, D], f32, name="t1", tag="t1")
                        nc.vector.tensor_mul(t1, xn, cos_sb[:BS, :])
                        nc.vector.tensor_mul(rot, rot, sin_sb[:BS, :])
                        nc.vector.tensor_add(dst[:, h * D : (h + 1) * D], t1, rot)

                # q01/k/v post-processing runs on ACT/DVE while PE continues
                norm_rope([q01_ps[:, 0:D], q01_ps[:, D : 2 * D]], qr_sb, True)
                norm_rope([k_ps], kr_sb, False)
                nc.vector.tensor_copy(v_sb, v_ps)
                norm_rope(
                    [q23_ps[:, 0:D], q23_ps[:, D : 2 * D]], qr_sb, True, h0=2
                )

                with tc.tile_pool(name="psT", bufs=2, space="PSUM") as psT:
                    for h in range(G):
                        tp = psT.tile([128, BS], f32, name="tp", tag="tp")
                        nc.tensor.transpose(
                            tp, qr_sb[:, h * D : (h + 1) * D], ident[:BS, :BS]
                        )
                        nc.vector.tensor_copy(qT_sb[:, h * BS : (h + 1) * BS], tp)
                    tpk = psT.tile([128, BS], f32, name="tpk", tag="tp")
                    nc.tensor.transpose(tpk, kr_sb, ident[:BS, :BS])
                    nc.vector.tensor_copy(kT_sb, tpk)

            qT3 = qT_sb.rearrange("p (h c) -> p h c", c=BS)

            EXPB = 16  # token tiles per exp batch
            attn3 = attn_T.rearrange("p (h c) -> p h c", c=BS)
            # greedy byte-balancing across the two HWDGE queues (sync, scalar)
            qload = {"sync": 0.0, "scalar": 0.0}

            def qpick(nbytes):
                e = min(qload, key=lambda k: qload[k])
                qload[e] += nbytes
                return nc.sync if e == "sync" else nc.scalar

            with tc.tile_pool(name="psB", bufs=1, space="PSUM") as psB:
                for gi, grp in enumerate(groups):
                    if gi == 2:
                        wsplit(wo_sb, wo[:, :], G * HID)
                    gnb = sum(nb[b] for b in grp)
                    gbase = pstart[grp[0]]
                    # one contiguous DMA per group for K and for V
                    ktg = sb.tile(
                        [128, gnb * BLOCK_SIZE], bf16, name=f"ktg{gi}",
                        tag="ktall", bufs=4,
                    )
                    vng = sb.tile(
                        [128, gnb * VW], bf16, name=f"vng{gi}", tag="vnat", bufs=4
                    )
                    qpick(gnb * BLOCK_SIZE * 128 * 2).dma_start(
                        out=ktg,
                        in_=kp[:, gbase * BLOCK_SIZE : (gbase + gnb) * BLOCK_SIZE],
                    )
                    qpick(gnb * VW * 128 * 2).dma_start(
                        out=vng, in_=vp[:, gbase * VW : (gbase + gnb) * VW]
                    )
                    for b in grp:
                        nbb = nb[b]
                        loc = pstart[b] - gbase  # block offset within group
                        T = int(seq_lens[b])
                        r = (T - 1) % BLOCK_SIZE
                        tmod = T % BLOCK_SIZE
                        kt = ktg[:, loc * BLOCK_SIZE : (loc + nbb) * BLOCK_SIZE]
                        vn = vng[:, loc * VW : (loc + nbb) * VW]
                        # new token's k/v replace the stale cache slot
                        col = (nbb - 1) * BLOCK_SIZE + r
                        nc.vector.tensor_copy(
                            kt[:, col : col + 1], kT_sb[:, b : b + 1]
                        )
                        nc.sync.dma_start(
                            out=vn[r : r + 1, (nbb - 1) * VW : (nbb - 1) * VW + D],
                            in_=v_sb[b : b + 1, :],
                        )
                        ot_ps = psB.tile(
                            [4, VW], f32, name=f"ot{b}", tag="ot", bufs=2
                        )
                        qTb = qT3[:, :, b]
                        ngrp = (nbb + EXPB - 1) // EXPB
                        for g in range(ngrp):
                            j0 = g * EXPB
                            w = min(EXPB, nbb - j0)
                            stp = psB.tile(
                                [128, 4 * EXPB], f32, name=f"stp{b}_{g}",
                                tag="stp", bufs=3,
                            )
                            for jj in range(w):
                                j = j0 + jj
                                nc.tensor.matmul(
                                    stp[:, 4 * jj : 4 * jj + 4],
                                    kt[:, j * BLOCK_SIZE : (j + 1) * BLOCK_SIZE],
                                    qTb,
                                    start=True,
                                    stop=True,
                                )
                            e = sb.tile(
                                [128, 4 * EXPB], bf16, name=f"e{b}_{g}", tag="e",
                                bufs=6,
                            )
                            if g == ngrp - 1 and tmod != 0:
                                if w > 1:
                                    nc.scalar.activation(
                                        e[:, 0 : 4 * (w - 1)],
                                        stp[:, 0 : 4 * (w - 1)], AF.Exp,
                                    )
                                nc.scalar.activation(
                                    e[:, 4 * (w - 1) : 4 * w],
                                    stp[:, 4 * (w - 1) : 4 * w], AF.Exp,
                                    bias=maskM[:, tmod : tmod + 1], scale=1.0,
                                )
                            else:
                                nc.scalar.activation(
                                    e[:, 0 : 4 * w], stp[:, 0 : 4 * w], AF.Exp
                                )
                            for jj in range(w):
                                j = j0 + jj
                                nc.tensor.matmul(
                                    ot_ps,
                                    e[:, 4 * jj : 4 * jj + 4],
                                    vn[:, j * VW : (j + 1) * VW],
                                    start=(j == 0),
                                    stop=(j == nbb - 1),
                                )
                        rec = smalls.tile([4, 1], f32, name=f"rec{b}", tag="rec")
                        nc.vector.reciprocal(rec, ot_ps[:, D : D + 1])
                        o_sb = smalls.tile([4, D], f32, name=f"o{b}", tag="o")
                        nc.vector.tensor_scalar_mul(o_sb, ot_ps[:, 0:D], rec)
                        tp2 = psB.tile(
                            [128, 4], f32, name=f"tp2_{b}", tag="tp2", bufs=2
                        )
                        nc.tensor.transpose(tp2, o_sb, ident[:4, :4])
                        nc.vector.tensor_copy(attn3[:, :, b], tp2)

            with tc.tile_pool(name="psC", bufs=2, space="PSUM") as psC:
                for c in range(HID // 512):
                    oc = psC.tile([BS, 512], f32, name=f"oc{c}", tag="oc")
                    for h in range(G):
                        nc.tensor.matmul(
                            oc,
                            attn_T[:, h * BS : (h + 1) * BS],
                            wo_sb[:, h * HID + c * 512 : h * HID + (c + 1) * 512],
                            start=(h == 0), stop=(h == G - 1),
                        )
                    ocs = sb.tile([BS, 512], f32, name=f"ocs{c}", tag="ocs", bufs=2)
                    nc.vector.tensor_copy(ocs, oc)
                    nc.sync.dma_start(out=outp[:, c * 512 : (c + 1) * 512], in_=ocs)

    nc.compile()
    return nc


def _pack_w(w):
    # [4096, C] -> [128, KTILES*C]; sbuf[p, t*C + c] == w[t*128 + p, c]
    C = w.shape[1]
    return np.ascontiguousarray(
        w.reshape(KTILES, 128, C).transpose(1, 0, 2).reshape(128, KTILES * C)
    ).astype(BF16)


def _prepare(inputs):
    hs = np.ascontiguousarray(np.asarray(inputs["hidden_states"], np.float32)[0])
    Wq = np.asarray(inputs["Wq"], np.float32)
    Wk = np.asarray(inputs["Wk"], np.float32)
    Wv = np.asarray(inputs["Wv"], np.float32)
    Wo = np.asarray(inputs["Wo"], np.float32)
    cos_t = np.ascontiguousarray(np.asarray(inputs["cos"], np.float32)[0])
    sin_t = np.ascontiguousarray(np.asarray(inputs["sin"], np.float32)[0])
    qnw = np.asarray(inputs["q_norm_w"], np.float32)
    knw = np.asarray(inputs["k_norm_w"], np.float32)
    key_cache = np.asarray(inputs["key_cache"], np.float32)
    value_cache = np.asarray(inputs["value_cache"], np.float32)
    seq_lens = np.asarray(inputs["seq_lens_k"]).astype(np.int64)
    bt = np.asarray(inputs["block_table"]).astype(np.int64)

    apply_qw = not np.all(qnw == 1.0)
    apply_kw = not np.all(knw == 1.0)

    nb, order, pstart, nbtot, groups = _plan(seq_lens)
    blocks = np.concatenate([bt[b, : nb[b]] for b in order])

    hidT = np.ascontiguousarray(
        hs.T.reshape(KTILES, 128, BS).transpose(1, 0, 2).reshape(128, KTILES * BS)
    ).astype(BF16)

    # K: [nbtot, tok, hkv, d] -> per head [d, nbtot*tok] bf16
    kg = key_cache[blocks]  # [nbtot, 128, HKV, 128]
    kall = np.ascontiguousarray(kg.transpose(2, 3, 0, 1)).astype(BF16)
    # V: per head [tok_in_block, nbtot*VW] with ones col at 128
    vg = value_cache[blocks]  # [nbtot, 128, HKV, 128]
    vall = np.empty((HKV, BLOCK_SIZE, nbtot, VW), BF16)
    vall[:, :, :, 0:D] = vg.transpose(2, 1, 0, 3)
    vall[:, :, :, D] = 1.0
    vall[:, :, :, D + 1] = 0.0

    miscd = np.zeros((128, 512), np.float32)
    miscd[:, 0:128] = np.where(
        np.arange(128)[:, None] < np.arange(128)[None, :], 0.0, -1e30
    )
    miscd[:, 128:256] = np.eye(128)
    miscd[0:BS, 256:384] = cos_t
    miscd[0:BS, 384:512] = sin_t

    in_maps = []
    for i in range(NCORES):
        m = {
            "hidT": hidT,
            "wq": np.concatenate(
                [
                    _pack_w(Wq[:, i * G * D : i * G * D + 2 * D]),
                    _pack_w(Wq[:, i * G * D + 2 * D : (i + 1) * G * D]),
                ],
                axis=1,
            ),
            "wkv": _pack_w(
                np.concatenate(
                    [Wk[:, i * D : (i + 1) * D], Wv[:, i * D : (i + 1) * D]], axis=1
                )
            ),
            "wo": np.ascontiguousarray(
                Wo[i * G * D : (i + 1) * G * D, :]
                .reshape(G, D, HID)
                .transpose(1, 0, 2)
                .reshape(128, G * HID)
            ).astype(BF16),
            "kp": np.ascontiguousarray(kall[i].reshape(128, nbtot * BLOCK_SIZE)),
            "vp": np.ascontiguousarray(vall[i].reshape(128, nbtot * VW)),
            "miscd": miscd,
        }
        if apply_qw:
            m["qw"] = np.ascontiguousarray(qnw.reshape(1, D))
        if apply_kw:
            m["kw"] = np.ascontiguousarray(knw.reshape(1, D))
        in_maps.append(m)

    key = (tuple(int(x) for x in seq_lens), apply_qw, apply_kw)
    if key not in _prog_cache:
        _prog_cache[key] = _build_program(seq_lens, apply_qw, apply_kw)
    nc = _prog_cache[key]
    return nc, in_maps


def kernel_with_stats(trace=False, **inputs):
    from concourse.bass_utils import run_bass_kernel_spmd

    nc, in_maps = _prepare(inputs)
    res = run_bass_kernel_spmd(
        nc, in_maps, core_ids=list(range(NCORES)), trace=trace
    )
    out = np.zeros((BS, HID), np.float32)
    for r in res.results:
        out += r["outp"]
    return out.reshape(1, BS, HID), res


def kernel(**inputs):
    out, _ = kernel_with_stats(trace=False, **inputs)
    return out
